# revision 10
# baseline (speedup 1.0000x reference)
"""LoRA self-attention TRN2 kernel (8 NeuronCores, SPMD) — v5.

Sharding: core c = (b, hp) with b = c // 4 (batch), hp = c % 4 (head group of
4 heads = 256 channels). Each core computes q/k/v projections (+LoRA) for its
256 output channels from the full x[b], runs attention for its 4 heads, and a
partial output projection over its 256 context channels. Host sums the 4
partials per batch element and adds bo.

Numerics: q/k projections and the [k,q]-oriented QK^T scores use bf16 hi/lo
splits (s = kh·qh + kl·qh + kh·ql, fp32-grade); the softmax shift m-hat comes
from a single-bf16 [q,k] score pass (error ≪ the exp-safety slack; the shift
cancels exactly in softmax). P·V and the output projection run in bf16.

v5 scheduling (the PE queue is strictly in-order, so long PE->DVE ping-pong
chains must be interleaved at fine grain with independent matmuls):
  - m-hat matmuls+reduces are emitted one per score-tile iteration (and
    threaded through the k/v projection loops for head 0), so the PE never
    idles behind a DVE reduce_max and the HAM clock gate stays warm.
  - x is DMA'd in ns-major 512-column slices, after the q-projection weights,
    so the first projection group starts ~8us in instead of ~27us.
  - v is computed directly in [T, O] orientation (no PE transposes).
  - the output projection + its DMA are interleaved into head 3's score loop
    (reusing the idle m-hat PSUM banks), removing the serial tail.
  - PV accumulators are evacuated to SBUF immediately so the next PV group
    never stalls behind the softmax-normalize chain.
  - ones-column on V makes PV row 64 the softmax normalizer Z (no reduce);
    m-hat lands in qla row 64 via a DRAM transpose bounce so the K=65 score
    matmul subtracts it inside PSUM for free.
  - when every LoRA B factor is zero (standard LoRA init), a specialized
    no-LoRA program is compiled and used; the general path handles B != 0.
"""
import sys

sys.path.insert(0, "/opt/trn_rl_repo")

from contextlib import ExitStack

import numpy as np
import ml_dtypes

import concourse.bass as bass
import concourse.tile as tile
from concourse import bacc, mybir
from concourse.bass import ts
from concourse.bass_utils import run_bass_kernel_spmd

F32 = mybir.dt.float32
BF16 = mybir.dt.bfloat16
bf16 = ml_dtypes.bfloat16
AX = mybir.AxisListType
Exp = mybir.ActivationFunctionType.Exp

T = 2048          # sequence length
E = 1024          # embed
OL = 256          # local output channels (4 heads)
D = 64            # head dim
NH = 4            # local heads
R = 8             # lora rank
CI = 8            # contraction chunks of 128 over E
NS = 4            # 512-wide slices over T
TC = 16           # 128-wide tiles over T
VW = 65           # v-aug width per head (64 + ones column)

_CACHE = {}


def _build(lora=True):
    key = ("nc", lora)
    if key in _CACHE:
        return _CACHE[key]

    nc = bacc.Bacc("TRN2", target_bir_lowering=False, debug=False)

    # ---- DRAM I/O ----
    xth_d = nc.dram_tensor("xth", [E, T], BF16, kind="ExternalInput")
    xtl_d = nc.dram_tensor("xtl", [E, T], BF16, kind="ExternalInput")
    w_d = {}
    for p in "qkv":
        for s in "hl":
            if p == "v" and s == "l":
                continue
            w_d[p + s] = nc.dram_tensor(f"w{p}{s}", [E, OL], BF16, kind="ExternalInput")
    woT_d = nc.dram_tensor("woT", [OL, E], BF16, kind="ExternalInput")
    if lora:
        ah_d = nc.dram_tensor("ah", [E, 3 * R], BF16, kind="ExternalInput")
        al_d = nc.dram_tensor("al", [E, 3 * R], BF16, kind="ExternalInput")
        b_d = {}
        for p in "qkv":
            for s in "hl":
                b_d[p + s] = nc.dram_tensor(f"b{p}{s}", [R, OL], BF16,
                                            kind="ExternalInput")
    ident_d = nc.dram_tensor("ident", [128, 128], BF16, kind="ExternalInput")
    outp_d = nc.dram_tensor("outp", [T, E], F32, kind="ExternalOutput")

    with tile.TileContext(nc) as tc, ExitStack() as ctx:
        # ---------------- persistent tiles ----------------
        # Per-head score operand layouts:
        #   khl[h] [128,T]: rows 0:64 = kT_hi(h), rows 64:128 = kT_lo(h)
        #   kha[h] [65,T]:  rows 0:64 = kT_hi(h), row 64 = ones
        #   qhh[h] [128,T]: rows 0:64 = qT_hi(h), rows 64:128 = qT_hi(h) (dup)
        #   qla[h] [65,T]:  rows 0:64 = qT_lo(h), row 64 = -m-hat
        pers = ctx.enter_context(tc.tile_pool(name="pers", bufs=1))
        khl = [pers.tile([128, T], BF16, name=f"khl{h}") for h in range(NH)]
        kha = [pers.tile([65, T], BF16, name=f"kha{h}") for h in range(NH)]
        qhh = [pers.tile([128, T], BF16, name=f"qhh{h}") for h in range(NH)]
        qla = [pers.tile([65, T], BF16, name=f"qla{h}") for h in range(NH)]
        v16 = [pers.tile([128, NH * VW], BF16, name=f"v16_{i}") for i in range(TC)]
        ident = pers.tile([128, 128], BF16, name="ident")
        ctxT_t = [pers.tile([128, T], BF16, name=f"ctxT{c}") for c in range(2)]

        # ---------------- attention-lifetime pools ----------------
        # (ptp/ost_p enter after phase 1 so their SBUF reuses the x tiles')
        att = ctx.enter_context(tc.tile_pool(name="att", bufs=2))
        drp = ctx.enter_context(tc.tile_pool(name="drp", bufs=2, space="DRAM"))
        sps = ctx.enter_context(
            tc.tile_pool(name="sps", bufs=2 if lora else 3, space="PSUM"))
        msp = ctx.enter_context(tc.tile_pool(name="msp", bufs=2, space="PSUM"))
        cps = ctx.enter_context(tc.tile_pool(name="cps", bufs=1, space="PSUM"))

        nc.sync.dma_start(out=ident, in_=ident_d[:, :])
        woT_t = []
        for cc in range(2):
            t_ = pers.tile([128, E], BF16, name=f"woT{cc}")
            nc.sync.dma_start(out=t_, in_=woT_d[ts(cc, 128), :])
            woT_t.append(t_)

        # ---------------- m-hat machinery (emitted interleaved) ----------
        # mh_step(h, i), i in 0..63: one single-bf16 [q,k] score matmul
        # (qt = i//4 stationary, k-slice i%4 moving) + row-max reduce.
        # mh_finish(h): merge quarter maxes, negate, PE-transpose, DRAM
        # bounce into qla[h] row 64.
        rm4 = {}

        def mh_step(h, i):
            qt, quarter = i // 4, i % 4
            if quarter == 0 and qt == 0:
                rm4[h] = [att.tile([128, 16], F32, tag=f"rm4{q}", name=f"rm4_{h}{q}")
                          for q in range(4)]
            ms = msp.tile([128, 512], F32, tag="ms", name="ms")
            nc.tensor.matmul(ms, qhh[h][0:64, ts(qt, 128)],
                             khl[h][0:64, ts(quarter, 512)], start=True, stop=True)
            nc.vector.reduce_max(out=rm4[h][quarter][:, qt:qt + 1], in_=ms, axis=AX.X)

        def mh_finish(h):
            r = rm4[h]
            ra = att.tile([128, 16], F32, name="ra")
            rb = att.tile([128, 16], F32, name="rb")
            nc.vector.tensor_max(ra, r[0], r[1])
            nc.vector.tensor_max(rb, r[2], r[3])
            rm16 = att.tile([128, 16], F32, name="rm16")
            nc.vector.tensor_max(rm16, ra, rb)
            rm16s = att.tile([128, 16], BF16, name="rm16s")
            nc.vector.tensor_scalar_mul(rm16s, rm16, -1.0)
            # transpose on PE, then a burst-contiguous DRAM bounce:
            # qla[h][64, qt*128+q] = rm16s[q, qt]
            mtr = msp.tile([16, 128], BF16, tag="ms", name="mtr")
            nc.tensor.transpose(mtr, rm16s, ident)
            rmT = att.tile([16, 128], BF16, name="rmT")
            nc.vector.tensor_copy(rmT, mtr)
            dr = drp.tile([16, 128], BF16, name="mh_dr")
            nc.sync.dma_start(out=dr, in_=rmT)
            src = bass.AP(tensor=dr.tensor, offset=dr.offset, ap=[[1, 16 * 128]])
            nc.sync.dma_start(out=qla[h][64:65, :], in_=src)

        # ---------------- phase 1: projections ----------------
        with ExitStack() as ph1:
            ld = ph1.enter_context(tc.tile_pool(name="ld", bufs=1))
            wpool = ph1.enter_context(tc.tile_pool(name="wpool", bufs=2))
            pps = ph1.enter_context(
                tc.tile_pool(name="pps", bufs=1 if lora else 2, space="PSUM"))
            if lora:
                upsp = ph1.enter_context(
                    tc.tile_pool(name="upsp", bufs=1, space="PSUM"))

            for h in range(NH):
                nc.vector.memset(kha[h][64:65, :], 1.0)
            for tci in range(TC):
                nc.vector.memset(v16[tci], 1.0)

            # q-projection weights first so the first MM group starts early
            wt = {}

            def load_w(p):
                wh_t, wl_t = [], []
                for ci in range(CI):
                    t_ = wpool.tile([128, OL], BF16, tag=f"wh{ci}", name=f"wh{ci}")
                    nc.sync.dma_start(out=t_, in_=w_d[p + "h"][ts(ci, 128), :])
                    wh_t.append(t_)
                    if p != "v":
                        t_ = wpool.tile([128, OL], BF16, tag=f"wl{ci}", name=f"wl{ci}")
                        nc.sync.dma_start(out=t_, in_=w_d[p + "l"][ts(ci, 128), :])
                        wl_t.append(t_)
                wt[p] = (wh_t, wl_t)

            load_w("q")

            # x arrives in ns-major 512-column slices so projection groups
            # can start before the full 8MB load lands
            xth_t = [ld.tile([128, T], BF16, name=f"xth{ci}") for ci in range(CI)]
            xtl_t = [ld.tile([128, T], BF16, name=f"xtl{ci}") for ci in range(CI)]
            for ns in range(NS):
                sl = ts(ns, 512)
                for ci in range(CI):
                    nc.sync.dma_start(out=xth_t[ci][:, sl], in_=xth_d[ts(ci, 128), sl])
                    nc.sync.dma_start(out=xtl_t[ci][:, sl], in_=xtl_d[ts(ci, 128), sl])

            u_bf = {}
            b_t = {}
            if lora:
                ah_t, al_t = [], []
                for ci in range(CI):
                    t_ = ld.tile([128, 3 * R], BF16, name=f"ah{ci}")
                    nc.sync.dma_start(out=t_, in_=ah_d[ts(ci, 128), :])
                    ah_t.append(t_)
                    t_ = ld.tile([128, 3 * R], BF16, name=f"al{ci}")
                    nc.sync.dma_start(out=t_, in_=al_d[ts(ci, 128), :])
                    al_t.append(t_)
                for key2, d in b_d.items():
                    t_ = ld.tile([R, OL], BF16, name=f"b{key2}")
                    nc.sync.dma_start(out=t_, in_=d[:, :])
                    b_t[key2] = t_

                # u_all = x @ A_all (split3), shared M=24 pass
                uf = ld.tile([3 * R, T], F32, name="uf")
                for ns in range(NS):
                    sl = ts(ns, 512)
                    ups = upsp.tile([3 * R, 512], F32, tag="ups", name="ups")
                    n_mm = 3 * CI
                    i = 0
                    for ci in range(CI):
                        for a_t, x_t in ((ah_t[ci], xth_t[ci]), (ah_t[ci], xtl_t[ci]),
                                         (al_t[ci], xth_t[ci])):
                            nc.tensor.matmul(ups, a_t, x_t[:, sl],
                                             start=(i == 0), stop=(i == n_mm - 1))
                            i += 1
                    nc.any.tensor_copy(uf[:, sl], ups)
                for pi, p in enumerate("qkv"):
                    upf = ld.tile([R, T], F32, tag="upf", name=f"u{p}f")
                    nc.sync.dma_start(out=upf, in_=uf[pi * R:(pi + 1) * R, :])
                    uh = ld.tile([R, T], BF16, name=f"u{p}h")
                    ul = ld.tile([R, T], BF16, name=f"u{p}l")
                    nc.vector.tensor_copy(uh, upf)
                    nc.vector.tensor_sub(ul, upf, uh)
                    u_bf[p + "h"], u_bf[p + "l"] = uh, ul

            # --- q/k projections, transposed layout [OL, T] ---
            def qk_proj(p, oc, mh_after=None):
                wh_t, wl_t = wt[p]
                osl = ts(oc, 128)
                h0, h1 = 2 * oc, 2 * oc + 1
                for ns in range(NS):
                    sl = ts(ns, 512)
                    ps = pps.tile([128, 512], F32, tag="proj", name="proj")
                    seq = []
                    for ci in range(CI):
                        seq += [(wh_t[ci][:, osl], xth_t[ci][:, sl]),
                                (wh_t[ci][:, osl], xtl_t[ci][:, sl]),
                                (wl_t[ci][:, osl], xth_t[ci][:, sl])]
                    if lora:
                        seq += [(b_t[p + "h"][:, osl], u_bf[p + "h"][:, sl]),
                                (b_t[p + "h"][:, osl], u_bf[p + "l"][:, sl]),
                                (b_t[p + "l"][:, osl], u_bf[p + "h"][:, sl])]
                    for i, (a, b_) in enumerate(seq):
                        nc.tensor.matmul(ps, a, b_, start=(i == 0),
                                         stop=(i == len(seq) - 1))
                    if p == "q":
                        for h, rows in ((h0, ps[0:64, :]), (h1, ps[64:128, :])):
                            nc.any.tensor_copy(qhh[h][0:64, sl], rows)
                            nc.any.tensor_copy(qhh[h][64:128, sl], rows)
                            nc.vector.tensor_sub(qla[h][0:64, sl], rows,
                                                 qhh[h][0:64, sl])
                    else:
                        for h, rows in ((h0, ps[0:64, :]), (h1, ps[64:128, :])):
                            nc.any.tensor_copy(khl[h][0:64, sl], rows)
                            nc.any.tensor_copy(kha[h][0:64, sl], rows)
                            nc.vector.tensor_sub(khl[h][64:128, sl], rows,
                                                 khl[h][0:64, sl])
                    if mh_after is not None:
                        h_mh, base = mh_after
                        for j in range(8):
                            mh_step(h_mh, base + ns * 8 + j)

            qk_proj("q", 0)
            load_w("k")
            qk_proj("q", 1)
            qk_proj("k", 0)
            load_w("v")
            # mh(0) needs qhh[0]/khl[0] (ready after q/k oc0): interleave its
            # 64 steps through k-oc1 (32) and the v loop (32)
            qk_proj("k", 1, mh_after=(0, 0))

            # --- v directly in [T, O] orientation (no transposes) ---
            wvh = wt["v"][0]
            for tci in range(TC):
                tsl = ts(tci, 128)
                ps = pps.tile([128, OL], F32, tag="proj", name="proj")
                seq = [(xth_t[ci][:, tsl], wvh[ci][:, :]) for ci in range(CI)]
                if lora:
                    seq += [(u_bf["vh"][:, tsl], b_t["vh"][:, :]),
                            (u_bf["vl"][:, tsl], b_t["vh"][:, :]),
                            (u_bf["vh"][:, tsl], b_t["vl"][:, :])]
                for i, (a, b_) in enumerate(seq):
                    nc.tensor.matmul(ps, a, b_, start=(i == 0),
                                     stop=(i == len(seq) - 1))
                for h in range(NH):
                    nc.any.tensor_copy(v16[tci][:, h * VW:h * VW + 64],
                                       ps[:, h * 64:(h + 1) * 64])
                for j in range(2):
                    mh_step(0, 32 + tci * 2 + j)
            mh_finish(0)

        # ---------------- phase 3: attention ----------------
        ptp = ctx.enter_context(tc.tile_pool(name="ptp", bufs=2))
        ost_p = ctx.enter_context(tc.tile_pool(name="ost", bufs=3))

        # outproj(tci): emitted interleaved into head 3's loop
        ops_state = {}

        def outproj_mm(tci, no):
            tsl = ts(tci, 128)
            op_t = msp.tile([128, 512], F32, tag="ms", name="op")
            for cc in range(2):
                nc.tensor.matmul(op_t, ctxT_t[cc][:, tsl], woT_t[cc][:, ts(no, 512)],
                                 start=(cc == 0), stop=(cc == 1))
            if no == 0:
                ops_state[tci] = ost_p.tile([128, E], F32, tag="ost", name="ost")
            ost = ops_state[tci]
            nc.vector.tensor_copy(ost[:, ts(no, 512)], op_t)
            if no == 1:
                nc.sync.dma_start(out=outp_d[tsl, :], in_=ost)

        def outproj_steps(qb):
            # 8 paired-MM slots per score loop: 4 tci x 2 no
            return [(tci, no) for tci in range(qb * 4, qb * 4 + 4)
                    for no in range(2)]

        for h in range(NH):
            ch = h // 2
            pr = (h % 2) * 64
            for qb in range(NS):
                qsl = ts(qb, 512)
                # --- sT pass: K-stacked scores with fused -m-hat -> exp ---
                # one mh(h+1) step (or outproj MM for h==3) per kt so the PE
                # queue never stalls behind the DVE reduce chain
                steps = outproj_steps(qb - 1) if (h == 3 and qb > 0) else None
                pT = [ptp.tile([128, 512], BF16, tag=f"pt{i}", name=f"pt{i}")
                      for i in range(TC)]
                for kt in range(TC):
                    st = sps.tile([128, 512], F32, tag="st", name="st")
                    # kh·qh + kl·qh in one K=128 matmul (qh duplicated)
                    nc.tensor.matmul(st, khl[h][:, ts(kt, 128)], qhh[h][:, qsl],
                                     start=True, stop=False)
                    # kh·ql + ones·(-m-hat), K=65
                    nc.tensor.matmul(st, kha[h][:, ts(kt, 128)],
                                     qla[h][:, qsl], start=False, stop=True)
                    nc.scalar.activation(out=pT[kt], in_=st, func=Exp, scale=0.125)
                    if h < NH - 1:
                        mh_step(h + 1, qb * TC + kt)
                    elif steps is not None and kt % 2 == 0:
                        outproj_mm(*steps[kt // 2])
                if h < NH - 1 and qb == NS - 1:
                    mh_finish(h + 1)
                # --- PV with ones column ---
                cxa = cps.tile([VW, 512], F32, tag="cxa", name="cxa")
                for kt in range(TC):
                    nc.tensor.matmul(cxa, v16[kt][:, h * VW:(h + 1) * VW], pT[kt],
                                     start=(kt == 0), stop=(kt == TC - 1))
                # evacuate PSUM immediately so the next PV group never waits
                cxs = att.tile([VW, 512], F32, tag="cxs", name="cxs")
                nc.vector.tensor_copy(cxs, cxa)
                # --- normalize by Z (row 64) off the critical path ---
                zrow = att.tile([1, 512], F32, name="zrow")
                nc.vector.tensor_copy(zrow, cxs[64:65, :])
                z_bc = att.tile([64, 512], F32, name="z_bc")
                nc.gpsimd.partition_broadcast(z_bc, zrow, channels=64)
                rcp_bc = att.tile([64, 512], F32, name="rcp_bc")
                nc.vector.reciprocal_approx_fast(out=rcp_bc, in_=z_bc)
                nc.vector.tensor_mul(ctxT_t[ch][pr:pr + 64, qsl], cxs[0:64, :],
                                     rcp_bc)

        # ---------------- tail: last output-projection block ----------------
        for tci, no in outproj_steps(NS - 1):
            outproj_mm(tci, no)

    nc.compile()
    _CACHE[key] = nc
    return nc


def _split(a):
    h = a.astype(bf16)
    l = (a - h.astype(np.float32)).astype(bf16)
    return h, l


def _shard(inputs, lora):
    x = np.asarray(inputs["x"], np.float32)
    Wo = np.asarray(inputs["Wo"], np.float32)
    ident = np.eye(128, dtype=np.float32).astype(bf16)
    if lora:
        A_all = np.concatenate([np.asarray(inputs["Aq"], np.float32),
                                np.asarray(inputs["Ak"], np.float32),
                                np.asarray(inputs["Av"], np.float32)], axis=1)
        ah, al = _split(A_all)
    in_maps = []
    for core in range(8):
        b, hp = core // 4, core % 4
        o0 = hp * OL
        xT = np.ascontiguousarray(x[b].T)
        xh, xl = _split(xT)
        m = {"xth": xh, "xtl": xl, "ident": ident}
        for p in "qkv":
            W = np.asarray(inputs["W" + p], np.float32)
            Ws = np.ascontiguousarray(W[o0:o0 + OL, :].T)
            wh, wl = _split(Ws)
            m["w%sh" % p] = wh
            if p != "v":
                m["w%sl" % p] = wl
            if lora:
                B = np.asarray(inputs["B" + p], np.float32)[:, o0:o0 + OL] * 2.0
                m["b%sh" % p], m["b%sl" % p] = _split(B)
        m["woT"] = np.ascontiguousarray(Wo[:, o0:o0 + OL].T).astype(bf16)
        if lora:
            m["ah"], m["al"] = ah, al
        in_maps.append(m)
    return in_maps


def _run(inputs, trace=False, **kw):
    lora = not all(
        np.count_nonzero(np.asarray(inputs["B" + p])) == 0 for p in "qkv")
    nc = _build(lora)
    in_maps = _shard(inputs, lora)
    res = run_bass_kernel_spmd(nc, in_maps, core_ids=list(range(8)), trace=trace, **kw)
    bo = np.asarray(inputs["bo"], np.float32)
    parts = [res.results[c]["outp"].astype(np.float64) for c in range(8)]
    out = np.stack([sum(parts[0:4]), sum(parts[4:8])]) + bo.astype(np.float64)
    return out.astype(np.float32), res


def kernel(**inputs):
    out, _ = _run(inputs)
    return out


# revision 17
# speedup vs baseline: 1.1991x; 1.1991x over previous
"""LoRA self-attention TRN2 kernel (8 NeuronCores, SPMD) — v5.

Sharding: core c = (b, hp) with b = c // 4 (batch), hp = c % 4 (head group of
4 heads = 256 channels). Each core computes q/k/v projections (+LoRA) for its
256 output channels from the full x[b], runs attention for its 4 heads, and a
partial output projection over its 256 context channels. Host sums the 4
partials per batch element and adds bo.

Numerics: q/k projections and the [k,q]-oriented QK^T scores use bf16 hi/lo
splits (s = kh·qh + kl·qh + kh·ql, fp32-grade); the softmax shift m-hat comes
from a single-bf16 [q,k] score pass (error ≪ the exp-safety slack; the shift
cancels exactly in softmax). P·V and the output projection run in bf16.

v5 scheduling (the PE queue is strictly in-order, so long PE->DVE ping-pong
chains must be interleaved at fine grain with independent matmuls):
  - m-hat matmuls+reduces are emitted one per score-tile iteration (and
    threaded through the k/v projection loops for head 0), so the PE never
    idles behind a DVE reduce_max and the HAM clock gate stays warm.
  - x is DMA'd in ns-major 512-column slices, after the q-projection weights,
    so the first projection group starts ~8us in instead of ~27us.
  - v is computed directly in [T, O] orientation (no PE transposes).
  - the output projection + its DMA are interleaved into head 3's score loop
    (reusing the idle m-hat PSUM banks), removing the serial tail.
  - PV accumulators are evacuated to SBUF immediately so the next PV group
    never stalls behind the softmax-normalize chain.
  - ones-column on V makes PV row 64 the softmax normalizer Z (no reduce);
    m-hat lands in qla row 64 via a DRAM transpose bounce so the K=65 score
    matmul subtracts it inside PSUM for free.
  - when every LoRA B factor is zero (standard LoRA init), a specialized
    no-LoRA program is compiled and used; the general path handles B != 0.
"""
import sys

sys.path.insert(0, "/opt/trn_rl_repo")

from contextlib import ExitStack

import numpy as np
import ml_dtypes

import concourse.bass as bass
import concourse.tile as tile
from concourse import bacc, mybir
from concourse.bass import ts
from concourse.bass_utils import run_bass_kernel_spmd

F32 = mybir.dt.float32
BF16 = mybir.dt.bfloat16
bf16 = ml_dtypes.bfloat16
AX = mybir.AxisListType
Exp = mybir.ActivationFunctionType.Exp

T = 2048          # sequence length
E = 1024          # embed
OL = 256          # local output channels (4 heads)
D = 64            # head dim
NH = 4            # local heads
R = 8             # lora rank
CI = 8            # contraction chunks of 128 over E
NS = 4            # 512-wide slices over T
TC = 16           # 128-wide tiles over T
VW = 65           # v-aug width per head (64 + ones column)

_CACHE = {}


def _build(lora=True):
    key = ("nc", lora)
    if key in _CACHE:
        return _CACHE[key]

    nc = bacc.Bacc("TRN2", target_bir_lowering=False, debug=False)

    # ---- DRAM I/O ----
    xth_d = nc.dram_tensor("xth", [E, T], BF16, kind="ExternalInput")
    xtl_d = nc.dram_tensor("xtl", [E, T], BF16, kind="ExternalInput")
    w_d = {}
    for p in "qkv":
        for s in "hl":
            if p == "v" and s == "l":
                continue
            w_d[p + s] = nc.dram_tensor(f"w{p}{s}", [E, OL], BF16, kind="ExternalInput")
    woT_d = nc.dram_tensor("woT", [OL, E], BF16, kind="ExternalInput")
    if lora:
        ah_d = nc.dram_tensor("ah", [E, 3 * R], BF16, kind="ExternalInput")
        al_d = nc.dram_tensor("al", [E, 3 * R], BF16, kind="ExternalInput")
        b_d = {}
        for p in "qkv":
            for s in "hl":
                b_d[p + s] = nc.dram_tensor(f"b{p}{s}", [R, OL], BF16,
                                            kind="ExternalInput")
    ident_d = nc.dram_tensor("ident", [128, 128], BF16, kind="ExternalInput")
    outp_d = nc.dram_tensor("outp", [T, E], F32, kind="ExternalOutput")

    with tile.TileContext(nc) as tc, ExitStack() as ctx:
        # ---------------- persistent tiles ----------------
        # Per-head score operand layouts:
        #   khl[h] [128,T]: rows 0:64 = kT_hi(h), rows 64:128 = kT_lo(h)
        #   kha[h] [65,T]:  rows 0:64 = kT_hi(h), row 64 = ones
        #   qhh[h] [128,T]: rows 0:64 = qT_hi(h), rows 64:128 = qT_hi(h) (dup)
        #   qla[h] [65,T]:  rows 0:64 = qT_lo(h), row 64 = -m-hat
        pers = ctx.enter_context(tc.tile_pool(name="pers", bufs=1))
        khl = [pers.tile([128, T], BF16, name=f"khl{h}") for h in range(NH)]
        kha = [pers.tile([65, T], BF16, name=f"kha{h}") for h in range(NH)]
        qhh = [pers.tile([128, T], BF16, name=f"qhh{h}") for h in range(NH)]
        qla = [pers.tile([65, T], BF16, name=f"qla{h}") for h in range(NH)]
        v16 = [pers.tile([128, NH * VW], BF16, name=f"v16_{i}") for i in range(TC)]
        ident = pers.tile([128, 128], BF16, name="ident")
        ctxT_t = [pers.tile([128, T], BF16, name=f"ctxT{c}") for c in range(2)]

        # ---------------- attention-lifetime pools ----------------
        # (ptp/ost_p and the PSUM pools enter after phase 1 so their space
        # reuses the x tiles' / projection pools')
        att = ctx.enter_context(tc.tile_pool(name="att", bufs=2))
        drp = ctx.enter_context(tc.tile_pool(name="drp", bufs=2, space="DRAM"))

        nc.sync.dma_start(out=ident, in_=ident_d[:, :])
        woT_t = []
        for cc in range(2):
            t_ = pers.tile([128, E], BF16, name=f"woT{cc}")
            nc.sync.dma_start(out=t_, in_=woT_d[ts(cc, 128), :])
            woT_t.append(t_)

        # ---------------- m-hat machinery (emitted interleaved) ----------
        # mh_step(h, i, pool), i in 0..63: one single-bf16 [q,k] score matmul
        # (qt = i//4 stationary, k-slice i%4 moving) + row-max reduce.
        # mh_finish(h, pool): merge quarter maxes, negate, PE-transpose, DRAM
        # bounce into qla[h] row 64.
        rm4 = {}

        def mh_step(h, i, pool):
            qt, quarter = i // 4, i % 4
            if quarter == 0 and qt == 0:
                rm4[h] = [att.tile([128, 16], F32, tag=f"rm4{q}", name=f"rm4_{h}{q}")
                          for q in range(4)]
            ms = pool.tile([128, 512], F32, tag="ms", name="ms")
            nc.tensor.matmul(ms, qhh[h][0:64, ts(qt, 128)],
                             khl[h][0:64, ts(quarter, 512)], start=True, stop=True)
            nc.vector.reduce_max(out=rm4[h][quarter][:, qt:qt + 1], in_=ms, axis=AX.X)

        def mh_finish(h, pool):
            r = rm4[h]
            ra = att.tile([128, 16], F32, name="ra")
            rb = att.tile([128, 16], F32, name="rb")
            nc.vector.tensor_max(ra, r[0], r[1])
            nc.vector.tensor_max(rb, r[2], r[3])
            rm16 = att.tile([128, 16], F32, name="rm16")
            nc.vector.tensor_max(rm16, ra, rb)
            rm16s = att.tile([128, 16], BF16, name="rm16s")
            nc.vector.tensor_scalar_mul(rm16s, rm16, -1.0)
            # transpose on PE, then a burst-contiguous DRAM bounce:
            # qla[h][64, qt*128+q] = rm16s[q, qt]
            mtr = pool.tile([16, 128], BF16, tag="ms", name="mtr")
            nc.tensor.transpose(mtr, rm16s, ident)
            rmT = att.tile([16, 128], BF16, name="rmT")
            nc.vector.tensor_copy(rmT, mtr)
            dr = drp.tile([16, 128], BF16, name="mh_dr")
            nc.sync.dma_start(out=dr, in_=rmT)
            src = bass.AP(tensor=dr.tensor, offset=dr.offset, ap=[[1, 16 * 128]])
            nc.sync.dma_start(out=qla[h][64:65, :], in_=src)

        # ---------------- phase 1: projections ----------------
        with ExitStack() as ph1:
            ld = ph1.enter_context(tc.tile_pool(name="ld", bufs=1))
            wpool = ph1.enter_context(tc.tile_pool(name="wpool", bufs=2))
            pps = ph1.enter_context(
                tc.tile_pool(name="pps", bufs=1 if lora else 2, space="PSUM"))
            msp0 = ph1.enter_context(tc.tile_pool(name="msp0", bufs=4, space="PSUM"))
            if lora:
                upsp = ph1.enter_context(
                    tc.tile_pool(name="upsp", bufs=1, space="PSUM"))

            for h in range(NH):
                nc.vector.memset(kha[h][64:65, :], 1.0)
            for tci in range(TC):
                nc.vector.memset(v16[tci], 1.0)

            # one consolidated DMA per (weight, half): [128, ci, OL] pattern
            wt = {}

            def load_w(p):
                out = []
                for s in ("h", "l"):
                    if p == "v" and s == "l":
                        out.append(None)
                        continue
                    wa = wpool.tile([128, CI, OL], BF16, tag=f"w{s}", name=f"w{p}{s}")
                    dt_ = w_d[p + s]
                    src = bass.AP(tensor=dt_, offset=0,
                                  ap=[[OL, 128], [128 * OL, CI], [1, OL]])
                    nc.sync.dma_start(out=wa, in_=src)
                    out.append(wa)
                wt[p] = out

            load_w("q")

            # x arrives in ns-major 512-column slices (one DMA per slice
            # covering all 16 row-chunks) so projection groups start early
            # and the Sync queue isn't serialized by per-chunk DMA issue
            xth_a = ld.tile([128, CI, T], BF16, name="xth_a")
            xtl_a = ld.tile([128, CI, T], BF16, name="xtl_a")
            for ns in range(NS):
                for dst, dram in ((xth_a, xth_d), (xtl_a, xtl_d)):
                    src = bass.AP(tensor=dram, offset=ns * 512,
                                  ap=[[T, 128], [128 * T, CI], [1, 512]])
                    nc.sync.dma_start(out=dst[:, :, ts(ns, 512)], in_=src)
            xth_t = [xth_a[:, ci, :] for ci in range(CI)]
            xtl_t = [xtl_a[:, ci, :] for ci in range(CI)]

            u_bf = {}
            b_t = {}
            if lora:
                ah_t, al_t = [], []
                for ci in range(CI):
                    t_ = ld.tile([128, 3 * R], BF16, name=f"ah{ci}")
                    nc.sync.dma_start(out=t_, in_=ah_d[ts(ci, 128), :])
                    ah_t.append(t_)
                    t_ = ld.tile([128, 3 * R], BF16, name=f"al{ci}")
                    nc.sync.dma_start(out=t_, in_=al_d[ts(ci, 128), :])
                    al_t.append(t_)
                for key2, d in b_d.items():
                    t_ = ld.tile([R, OL], BF16, name=f"b{key2}")
                    nc.sync.dma_start(out=t_, in_=d[:, :])
                    b_t[key2] = t_

                # u_all = x @ A_all (split3), shared M=24 pass
                uf = ld.tile([3 * R, T], F32, name="uf")
                for ns in range(NS):
                    sl = ts(ns, 512)
                    ups = upsp.tile([3 * R, 512], F32, tag="ups", name="ups")
                    n_mm = 3 * CI
                    i = 0
                    for ci in range(CI):
                        for a_t, x_t in ((ah_t[ci], xth_t[ci]), (ah_t[ci], xtl_t[ci]),
                                         (al_t[ci], xth_t[ci])):
                            nc.tensor.matmul(ups, a_t, x_t[:, sl],
                                             start=(i == 0), stop=(i == n_mm - 1))
                            i += 1
                    nc.any.tensor_copy(uf[:, sl], ups)
                for pi, p in enumerate("qkv"):
                    upf = ld.tile([R, T], F32, tag="upf", name=f"u{p}f")
                    nc.sync.dma_start(out=upf, in_=uf[pi * R:(pi + 1) * R, :])
                    uh = ld.tile([R, T], BF16, name=f"u{p}h")
                    ul = ld.tile([R, T], BF16, name=f"u{p}l")
                    nc.vector.tensor_copy(uh, upf)
                    nc.vector.tensor_sub(ul, upf, uh)
                    u_bf[p + "h"], u_bf[p + "l"] = uh, ul

            # --- q/k projections, transposed layout [OL, T] ---
            def qk_proj(p, oc, mh_after=None):
                wh_a, wl_a = wt[p]
                osl = slice(oc * 128, oc * 128 + 128)
                h0, h1 = 2 * oc, 2 * oc + 1
                for ns in range(NS):
                    sl = ts(ns, 512)
                    ps = pps.tile([128, 512], F32, tag="proj", name="proj")
                    seq = []
                    for ci in range(CI):
                        seq += [(wh_a[:, ci, osl], xth_t[ci][:, sl]),
                                (wh_a[:, ci, osl], xtl_t[ci][:, sl]),
                                (wl_a[:, ci, osl], xth_t[ci][:, sl])]
                    if lora:
                        seq += [(b_t[p + "h"][:, osl], u_bf[p + "h"][:, sl]),
                                (b_t[p + "h"][:, osl], u_bf[p + "l"][:, sl]),
                                (b_t[p + "l"][:, osl], u_bf[p + "h"][:, sl])]
                    for i, (a, b_) in enumerate(seq):
                        nc.tensor.matmul(ps, a, b_, start=(i == 0),
                                         stop=(i == len(seq) - 1))
                    if p == "q":
                        for h, rows in ((h0, ps[0:64, :]), (h1, ps[64:128, :])):
                            nc.any.tensor_copy(qhh[h][0:64, sl], rows)
                            nc.any.tensor_copy(qhh[h][64:128, sl], rows)
                            nc.vector.tensor_sub(qla[h][0:64, sl], rows,
                                                 qhh[h][0:64, sl])
                    else:
                        for h, rows in ((h0, ps[0:64, :]), (h1, ps[64:128, :])):
                            nc.any.tensor_copy(khl[h][0:64, sl], rows)
                            nc.any.tensor_copy(kha[h][0:64, sl], rows)
                            nc.vector.tensor_sub(khl[h][64:128, sl], rows,
                                                 khl[h][0:64, sl])
                    if mh_after is not None:
                        h_mh, base = mh_after
                        for j in range(8):
                            mh_step(h_mh, base + ns * 8 + j, msp0)

            qk_proj("q", 0)
            load_w("k")
            qk_proj("q", 1)
            qk_proj("k", 0)
            load_w("v")
            # mh(0) needs qhh[0]/khl[0] (ready after q/k oc0): interleave its
            # 64 steps through k-oc1 (32) and the v loop (32)
            qk_proj("k", 1, mh_after=(0, 0))

            # --- v directly in [T, O] orientation (no transposes) ---
            wvh = wt["v"][0]
            for tci in range(TC):
                tsl = ts(tci, 128)
                ps = pps.tile([128, OL], F32, tag="proj", name="proj")
                seq = [(xth_t[ci][:, tsl], wvh[:, ci, :]) for ci in range(CI)]
                if lora:
                    seq += [(u_bf["vh"][:, tsl], b_t["vh"][:, :]),
                            (u_bf["vl"][:, tsl], b_t["vh"][:, :]),
                            (u_bf["vh"][:, tsl], b_t["vl"][:, :])]
                for i, (a, b_) in enumerate(seq):
                    nc.tensor.matmul(ps, a, b_, start=(i == 0),
                                     stop=(i == len(seq) - 1))
                for h in range(NH):
                    nc.any.tensor_copy(v16[tci][:, h * VW:h * VW + 64],
                                       ps[:, h * 64:(h + 1) * 64])
                for j in range(2):
                    mh_step(0, 32 + tci * 2 + j, msp0)
            mh_finish(0, msp0)

        # ---------------- phase 3: attention ----------------
        ptp = ctx.enter_context(tc.tile_pool(name="ptp", bufs=2))
        ost_p = ctx.enter_context(tc.tile_pool(name="ost", bufs=3))
        sps = ctx.enter_context(
            tc.tile_pool(name="sps", bufs=2 if lora else 3, space="PSUM"))
        msp = ctx.enter_context(tc.tile_pool(name="msp", bufs=4, space="PSUM"))
        cps = ctx.enter_context(tc.tile_pool(name="cps", bufs=1, space="PSUM"))

        # outproj(tci): emitted interleaved into head 3's loop
        ops_state = {}

        def outproj_mm(tci, no):
            tsl = ts(tci, 128)
            op_t = msp.tile([128, 512], F32, tag="ms", name="op")
            for cc in range(2):
                nc.tensor.matmul(op_t, ctxT_t[cc][:, tsl], woT_t[cc][:, ts(no, 512)],
                                 start=(cc == 0), stop=(cc == 1))
            if no == 0:
                ops_state[tci] = ost_p.tile([128, E], F32, tag="ost", name="ost")
            ost = ops_state[tci]
            nc.vector.tensor_copy(ost[:, ts(no, 512)], op_t)
            if no == 1:
                nc.sync.dma_start(out=outp_d[tsl, :], in_=ost)

        def outproj_steps(qb):
            # 8 paired-MM slots per score loop: 4 tci x 2 no
            return [(tci, no) for tci in range(qb * 4, qb * 4 + 4)
                    for no in range(2)]

        for h in range(NH):
            ch = h // 2
            pr = (h % 2) * 64
            for qb in range(NS):
                qsl = ts(qb, 512)
                # --- sT pass: K-stacked scores with fused -m-hat -> exp ---
                # one mh(h+1) step (or outproj MM for h==3) per kt so the PE
                # queue never stalls behind the DVE reduce chain
                steps = outproj_steps(qb - 1) if (h == 3 and qb > 0) else None
                pT = [ptp.tile([128, 512], BF16, tag=f"pt{i}", name=f"pt{i}")
                      for i in range(TC)]
                for kt in range(TC):
                    st = sps.tile([128, 512], F32, tag="st", name="st")
                    # kh·qh + kl·qh in one K=128 matmul (qh duplicated)
                    nc.tensor.matmul(st, khl[h][:, ts(kt, 128)], qhh[h][:, qsl],
                                     start=True, stop=False)
                    # kh·ql + ones·(-m-hat), K=65
                    nc.tensor.matmul(st, kha[h][:, ts(kt, 128)],
                                     qla[h][:, qsl], start=False, stop=True)
                    nc.scalar.activation(out=pT[kt], in_=st, func=Exp, scale=0.125)
                    if h < NH - 1:
                        mh_step(h + 1, qb * TC + kt, msp)
                    elif steps is not None and kt % 2 == 0:
                        outproj_mm(*steps[kt // 2])
                if h < NH - 1 and qb == NS - 1:
                    mh_finish(h + 1, msp)
                # --- PV with ones column ---
                cxa = cps.tile([VW, 512], F32, tag="cxa", name="cxa")
                for kt in range(TC):
                    nc.tensor.matmul(cxa, v16[kt][:, h * VW:(h + 1) * VW], pT[kt],
                                     start=(kt == 0), stop=(kt == TC - 1))
                # evacuate PSUM immediately so the next PV group never waits
                cxs = att.tile([VW, 512], F32, tag="cxs", name="cxs")
                nc.vector.tensor_copy(cxs, cxa)
                # --- normalize by Z (row 64) off the critical path ---
                zrow = att.tile([1, 512], F32, name="zrow")
                nc.vector.tensor_copy(zrow, cxs[64:65, :])
                z_bc = att.tile([64, 512], F32, name="z_bc")
                nc.gpsimd.partition_broadcast(z_bc, zrow, channels=64)
                rcp_bc = att.tile([64, 512], F32, name="rcp_bc")
                nc.vector.reciprocal_approx_fast(out=rcp_bc, in_=z_bc)
                nc.vector.tensor_mul(ctxT_t[ch][pr:pr + 64, qsl], cxs[0:64, :],
                                     rcp_bc)

        # ---------------- tail: last output-projection block ----------------
        for tci, no in outproj_steps(NS - 1):
            outproj_mm(tci, no)

    nc.compile()
    _CACHE[key] = nc
    return nc


def _split(a):
    h = a.astype(bf16)
    l = (a - h.astype(np.float32)).astype(bf16)
    return h, l


def _shard(inputs, lora):
    x = np.asarray(inputs["x"], np.float32)
    Wo = np.asarray(inputs["Wo"], np.float32)
    ident = np.eye(128, dtype=np.float32).astype(bf16)
    if lora:
        A_all = np.concatenate([np.asarray(inputs["Aq"], np.float32),
                                np.asarray(inputs["Ak"], np.float32),
                                np.asarray(inputs["Av"], np.float32)], axis=1)
        ah, al = _split(A_all)
    in_maps = []
    for core in range(8):
        b, hp = core // 4, core % 4
        o0 = hp * OL
        xT = np.ascontiguousarray(x[b].T)
        xh, xl = _split(xT)
        m = {"xth": xh, "xtl": xl, "ident": ident}
        for p in "qkv":
            W = np.asarray(inputs["W" + p], np.float32)
            Ws = np.ascontiguousarray(W[o0:o0 + OL, :].T)
            wh, wl = _split(Ws)
            m["w%sh" % p] = wh
            if p != "v":
                m["w%sl" % p] = wl
            if lora:
                B = np.asarray(inputs["B" + p], np.float32)[:, o0:o0 + OL] * 2.0
                m["b%sh" % p], m["b%sl" % p] = _split(B)
        m["woT"] = np.ascontiguousarray(Wo[:, o0:o0 + OL].T).astype(bf16)
        if lora:
            m["ah"], m["al"] = ah, al
        in_maps.append(m)
    return in_maps


def _run(inputs, trace=False, **kw):
    lora = not all(
        np.count_nonzero(np.asarray(inputs["B" + p])) == 0 for p in "qkv")
    nc = _build(lora)
    in_maps = _shard(inputs, lora)
    res = run_bass_kernel_spmd(nc, in_maps, core_ids=list(range(8)), trace=trace, **kw)
    bo = np.asarray(inputs["bo"], np.float32)
    parts = [res.results[c]["outp"].astype(np.float64) for c in range(8)]
    out = np.stack([sum(parts[0:4]), sum(parts[4:8])]) + bo.astype(np.float64)
    return out.astype(np.float32), res


def kernel(**inputs):
    out, _ = _run(inputs)
    return out


# revision 24
# speedup vs baseline: 1.3789x; 1.1499x over previous
"""LoRA self-attention TRN2 kernel (8 NeuronCores, SPMD) — v5.

Sharding: core c = (b, hp) with b = c // 4 (batch), hp = c % 4 (head group of
4 heads = 256 channels). Each core computes q/k/v projections (+LoRA) for its
256 output channels from the full x[b], runs attention for its 4 heads, and a
partial output projection over its 256 context channels. Host sums the 4
partials per batch element and adds bo.

Numerics: q/k projections and the [k,q]-oriented QK^T scores use bf16 hi/lo
splits (s = kh·qh + kl·qh + kh·ql, fp32-grade); the softmax shift m-hat comes
from a single-bf16 [q,k] score pass (error ≪ the exp-safety slack; the shift
cancels exactly in softmax). P·V and the output projection run in bf16.

v5 scheduling (the PE queue is strictly in-order, so long PE->DVE ping-pong
chains must be interleaved at fine grain with independent matmuls):
  - m-hat matmuls+reduces are emitted one per score-tile iteration (and
    threaded through the k/v projection loops for head 0), so the PE never
    idles behind a DVE reduce_max and the HAM clock gate stays warm.
  - x is DMA'd in ns-major 512-column slices, after the q-projection weights,
    so the first projection group starts ~8us in instead of ~27us.
  - v is computed directly in [T, O] orientation (no PE transposes).
  - the output projection + its DMA are interleaved into head 3's score loop
    (reusing the idle m-hat PSUM banks), removing the serial tail.
  - PV accumulators are evacuated to SBUF immediately so the next PV group
    never stalls behind the softmax-normalize chain.
  - ones-column on V makes PV row 64 the softmax normalizer Z (no reduce);
    m-hat lands in qla row 64 via a DRAM transpose bounce so the K=65 score
    matmul subtracts it inside PSUM for free.
  - when every LoRA B factor is zero (standard LoRA init), a specialized
    no-LoRA program is compiled and used; the general path handles B != 0.
"""
import sys

sys.path.insert(0, "/opt/trn_rl_repo")

from contextlib import ExitStack

import numpy as np
import ml_dtypes

import concourse.bass as bass
import concourse.tile as tile
from concourse import bacc, mybir
from concourse.bass import ts
from concourse.bass_utils import run_bass_kernel_spmd

F32 = mybir.dt.float32
BF16 = mybir.dt.bfloat16
bf16 = ml_dtypes.bfloat16
AX = mybir.AxisListType
Exp = mybir.ActivationFunctionType.Exp

T = 2048          # sequence length
E = 1024          # embed
OL = 256          # local output channels (4 heads)
D = 64            # head dim
NH = 4            # local heads
R = 8             # lora rank
CI = 8            # contraction chunks of 128 over E
NS = 4            # 512-wide slices over T
TC = 16           # 128-wide tiles over T
VW = 65           # v-aug width per head (64 + ones column)

_CACHE = {}


def _build(lora=True):
    key = ("nc", lora)
    if key in _CACHE:
        return _CACHE[key]

    nc = bacc.Bacc("TRN2", target_bir_lowering=False, debug=False)

    # ---- DRAM I/O ----
    xth_d = nc.dram_tensor("xth", [E, T], BF16, kind="ExternalInput")
    xtl_d = nc.dram_tensor("xtl", [E, T], BF16, kind="ExternalInput")
    w_d = {}
    for p in "qkv":
        for s in "hl":
            if p == "v" and s == "l":
                continue
            w_d[p + s] = nc.dram_tensor(f"w{p}{s}", [E, OL], BF16, kind="ExternalInput")
    woT_d = nc.dram_tensor("woT", [OL, E], BF16, kind="ExternalInput")
    if lora:
        ah_d = nc.dram_tensor("ah", [E, 3 * R], BF16, kind="ExternalInput")
        al_d = nc.dram_tensor("al", [E, 3 * R], BF16, kind="ExternalInput")
        b_d = {}
        for p in "qkv":
            for s in "hl":
                b_d[p + s] = nc.dram_tensor(f"b{p}{s}", [R, OL], BF16,
                                            kind="ExternalInput")
    ident_d = nc.dram_tensor("ident", [128, 128], BF16, kind="ExternalInput")
    outp_d = nc.dram_tensor("outp", [T, E], BF16, kind="ExternalOutput")

    with tile.TileContext(nc) as tc, ExitStack() as ctx:
        # ---------------- persistent tiles ----------------
        # Per-head score operand layouts:
        #   khl[h] [128,T]: rows 0:64 = kT_hi(h), rows 64:128 = kT_lo(h)
        #   kha[h] [65,T]:  rows 0:64 = kT_hi(h), row 64 = ones
        #   qhh[h] [128,T]: rows 0:64 = qT_hi(h), rows 64:128 = qT_hi(h) (dup)
        #   qla[h] [65,T]:  rows 0:64 = qT_lo(h), row 64 = -m-hat
        pers = ctx.enter_context(tc.tile_pool(name="pers", bufs=1))
        khl = [pers.tile([128, T], BF16, name=f"khl{h}") for h in range(NH)]
        kha = [pers.tile([65, T], BF16, name=f"kha{h}") for h in range(NH)]
        qhh = [pers.tile([128, T], BF16, name=f"qhh{h}") for h in range(NH)]
        qla = [pers.tile([65, T], BF16, name=f"qla{h}") for h in range(NH)]
        v16 = [pers.tile([128, NH * VW], BF16, name=f"v16_{i}") for i in range(TC)]
        ident = pers.tile([128, 128], BF16, name="ident")
        ctxT_t = [pers.tile([128, T], BF16, name=f"ctxT{c}") for c in range(2)]

        # ---------------- attention-lifetime pools ----------------
        # (ptp/ost_p and the PSUM pools enter after phase 1 so their space
        # reuses the x tiles' / projection pools')
        att = ctx.enter_context(tc.tile_pool(name="att", bufs=2))
        drp = ctx.enter_context(tc.tile_pool(name="drp", bufs=2, space="DRAM"))

        woT_t = [pers.tile([128, E], BF16, name=f"woT{cc}") for cc in range(2)]

        # ---------------- m-hat machinery (emitted interleaved) ----------
        # mh_step(h, i, pool), i in 0..63: one single-bf16 [q,k] score matmul
        # (qt = i//4 stationary, k-slice i%4 moving) + row-max reduce.
        # mh_finish(h, pool): merge quarter maxes, negate, PE-transpose, DRAM
        # bounce into qla[h] row 64.
        rm4 = {}

        def mh_step(h, i, pool):
            qt, quarter = i // 4, i % 4
            if quarter == 0 and qt == 0:
                rm4[h] = [att.tile([128, 16], F32, tag=f"rm4{q}", name=f"rm4_{h}{q}")
                          for q in range(4)]
            ms = pool.tile([128, 512], F32, tag="ms", name="ms")
            nc.tensor.matmul(ms, qhh[h][0:64, ts(qt, 128)],
                             khl[h][0:64, ts(quarter, 512)], start=True, stop=True)
            nc.vector.reduce_max(out=rm4[h][quarter][:, qt:qt + 1], in_=ms, axis=AX.X)

        def mh_finish(h, pool):
            r = rm4[h]
            ra = att.tile([128, 16], F32, name="ra")
            rb = att.tile([128, 16], F32, name="rb")
            nc.vector.tensor_max(ra, r[0], r[1])
            nc.vector.tensor_max(rb, r[2], r[3])
            rm16 = att.tile([128, 16], F32, name="rm16")
            nc.vector.tensor_max(rm16, ra, rb)
            rm16s = att.tile([128, 16], BF16, name="rm16s")
            nc.vector.tensor_scalar_mul(rm16s, rm16, -1.0)
            # transpose on PE, then a burst-contiguous DRAM bounce:
            # qla[h][64, qt*128+q] = rm16s[q, qt]
            mtr = pool.tile([16, 128], BF16, tag="ms", name="mtr")
            nc.tensor.transpose(mtr, rm16s, ident)
            rmT = att.tile([16, 128], BF16, name="rmT")
            nc.vector.tensor_copy(rmT, mtr)
            dr = drp.tile([16, 128], BF16, name="mh_dr")
            nc.sync.dma_start(out=dr, in_=rmT)
            src = bass.AP(tensor=dr.tensor, offset=dr.offset, ap=[[1, 16 * 128]])
            nc.sync.dma_start(out=qla[h][64:65, :], in_=src)

        # ---------------- phase 1: projections ----------------
        with ExitStack() as ph1:
            ld = ph1.enter_context(tc.tile_pool(name="ld", bufs=1))
            wpool = ph1.enter_context(tc.tile_pool(name="wpool", bufs=2))
            pps = ph1.enter_context(
                tc.tile_pool(name="pps", bufs=1 if lora else 2, space="PSUM"))
            msp0 = ph1.enter_context(tc.tile_pool(name="msp0", bufs=4, space="PSUM"))
            if lora:
                upsp = ph1.enter_context(
                    tc.tile_pool(name="upsp", bufs=1, space="PSUM"))

            for h in range(NH):
                nc.vector.memset(kha[h][64:65, :], 1.0)
            for tci in range(TC):
                nc.vector.memset(v16[tci], 1.0)

            # one consolidated DMA per (weight, half): [128, ci, OL] pattern
            wt = {}

            def load_w(p):
                out = []
                for s in ("h", "l"):
                    if p == "v" and s == "l":
                        out.append(None)
                        continue
                    wa = wpool.tile([128, CI, OL], BF16, tag=f"w{s}", name=f"w{p}{s}")
                    dt_ = w_d[p + s]
                    src = bass.AP(tensor=dt_, offset=0,
                                  ap=[[OL, 128], [128 * OL, CI], [1, OL]])
                    nc.sync.dma_start(out=wa, in_=src)
                    out.append(wa)
                wt[p] = out

            load_w("q")

            # x arrives in ns-major 512-column slices (one DMA per slice
            # covering all 16 row-chunks) so projection groups start early
            # and the Sync queue isn't serialized by per-chunk DMA issue
            xth_a = ld.tile([128, CI, T], BF16, name="xth_a")
            xtl_a = ld.tile([128, CI, T], BF16, name="xtl_a")
            for ns in range(NS):
                for dst, dram in ((xth_a, xth_d), (xtl_a, xtl_d)):
                    src = bass.AP(tensor=dram, offset=ns * 512,
                                  ap=[[T, 128], [128 * T, CI], [1, 512]])
                    nc.sync.dma_start(out=dst[:, :, ts(ns, 512)], in_=src)
            xth_t = [xth_a[:, ci, :] for ci in range(CI)]
            xtl_t = [xtl_a[:, ci, :] for ci in range(CI)]

            # ident / woT are not needed until late; don't delay x/w DMAs
            nc.sync.dma_start(out=ident, in_=ident_d[:, :])
            for cc in range(2):
                nc.sync.dma_start(out=woT_t[cc], in_=woT_d[ts(cc, 128), :])

            u_bf = {}
            b_t = {}
            if lora:
                ah_t, al_t = [], []
                for ci in range(CI):
                    t_ = ld.tile([128, 3 * R], BF16, name=f"ah{ci}")
                    nc.sync.dma_start(out=t_, in_=ah_d[ts(ci, 128), :])
                    ah_t.append(t_)
                    t_ = ld.tile([128, 3 * R], BF16, name=f"al{ci}")
                    nc.sync.dma_start(out=t_, in_=al_d[ts(ci, 128), :])
                    al_t.append(t_)
                for key2, d in b_d.items():
                    t_ = ld.tile([R, OL], BF16, name=f"b{key2}")
                    nc.sync.dma_start(out=t_, in_=d[:, :])
                    b_t[key2] = t_

                # u_all = x @ A_all (split3), shared M=24 pass
                uf = ld.tile([3 * R, T], F32, name="uf")
                for ns in range(NS):
                    sl = ts(ns, 512)
                    ups = upsp.tile([3 * R, 512], F32, tag="ups", name="ups")
                    n_mm = 3 * CI
                    i = 0
                    for ci in range(CI):
                        for a_t, x_t in ((ah_t[ci], xth_t[ci]), (ah_t[ci], xtl_t[ci]),
                                         (al_t[ci], xth_t[ci])):
                            nc.tensor.matmul(ups, a_t, x_t[:, sl],
                                             start=(i == 0), stop=(i == n_mm - 1))
                            i += 1
                    nc.any.tensor_copy(uf[:, sl], ups)
                for pi, p in enumerate("qkv"):
                    upf = ld.tile([R, T], F32, tag="upf", name=f"u{p}f")
                    nc.sync.dma_start(out=upf, in_=uf[pi * R:(pi + 1) * R, :])
                    uh = ld.tile([R, T], BF16, name=f"u{p}h")
                    ul = ld.tile([R, T], BF16, name=f"u{p}l")
                    nc.vector.tensor_copy(uh, upf)
                    nc.vector.tensor_sub(ul, upf, uh)
                    u_bf[p + "h"], u_bf[p + "l"] = uh, ul

            # --- q/k projections, transposed layout [OL, T] ---
            # MM sequence is kind-major (wh·xh first) so the first group can
            # start before xtl's DMA lands
            def qk_proj(p, oc, mh_per_ns=0, mh_base=0):
                wh_a, wl_a = wt[p]
                osl = slice(oc * 128, oc * 128 + 128)
                h0, h1 = 2 * oc, 2 * oc + 1
                for ns in range(NS):
                    sl = ts(ns, 512)
                    ps = pps.tile([128, 512], F32, tag="proj", name="proj")
                    seq = [(wh_a[:, ci, osl], xth_t[ci][:, sl]) for ci in range(CI)]
                    seq += [(wh_a[:, ci, osl], xtl_t[ci][:, sl]) for ci in range(CI)]
                    seq += [(wl_a[:, ci, osl], xth_t[ci][:, sl]) for ci in range(CI)]
                    if lora:
                        seq += [(b_t[p + "h"][:, osl], u_bf[p + "h"][:, sl]),
                                (b_t[p + "h"][:, osl], u_bf[p + "l"][:, sl]),
                                (b_t[p + "l"][:, osl], u_bf[p + "h"][:, sl])]
                    for i, (a, b_) in enumerate(seq):
                        nc.tensor.matmul(ps, a, b_, start=(i == 0),
                                         stop=(i == len(seq) - 1))
                    if p == "q":
                        for h, rows in ((h0, ps[0:64, :]), (h1, ps[64:128, :])):
                            nc.any.tensor_copy(qhh[h][0:64, sl], rows)
                            nc.any.tensor_copy(qhh[h][64:128, sl], rows)
                            nc.vector.tensor_sub(qla[h][0:64, sl], rows,
                                                 qhh[h][0:64, sl])
                    else:
                        for h, rows in ((h0, ps[0:64, :]), (h1, ps[64:128, :])):
                            nc.any.tensor_copy(khl[h][0:64, sl], rows)
                            nc.any.tensor_copy(kha[h][0:64, sl], rows)
                            nc.vector.tensor_sub(khl[h][64:128, sl], rows,
                                                 khl[h][0:64, sl])
                    for j in range(mh_per_ns):
                        mh_step(0, mh_base + ns * mh_per_ns + j, msp0)

            qk_proj("q", 0)
            load_w("k")
            qk_proj("k", 0)
            # mh(0) needs qhh[0]/khl[0] (ready after q/k oc0): spread its 64
            # steps through q-oc1 (32), k-oc1 (24) and v tci 0..7 (8) so the
            # DVE reduce chain finishes alongside the v projection
            qk_proj("q", 1, mh_per_ns=8, mh_base=0)
            qk_proj("k", 1, mh_per_ns=6, mh_base=32)
            load_w("v")

            # --- v directly in [T, O] orientation (no transposes) ---
            wvh = wt["v"][0]
            for tci in range(TC):
                tsl = ts(tci, 128)
                ps = pps.tile([128, OL], F32, tag="proj", name="proj")
                seq = [(xth_t[ci][:, tsl], wvh[:, ci, :]) for ci in range(CI)]
                if lora:
                    seq += [(u_bf["vh"][:, tsl], b_t["vh"][:, :]),
                            (u_bf["vl"][:, tsl], b_t["vh"][:, :]),
                            (u_bf["vh"][:, tsl], b_t["vl"][:, :])]
                for i, (a, b_) in enumerate(seq):
                    nc.tensor.matmul(ps, a, b_, start=(i == 0),
                                     stop=(i == len(seq) - 1))
                for h in range(NH):
                    nc.any.tensor_copy(v16[tci][:, h * VW:h * VW + 64],
                                       ps[:, h * 64:(h + 1) * 64])
                if tci < 8:
                    mh_step(0, 56 + tci, msp0)
                if tci == 8:
                    mh_finish(0, msp0)

        # ---------------- phase 3: attention ----------------
        ptp = ctx.enter_context(tc.tile_pool(name="ptp", bufs=2))
        ost_p = ctx.enter_context(tc.tile_pool(name="ost", bufs=3))
        sps = ctx.enter_context(
            tc.tile_pool(name="sps", bufs=2 if lora else 3, space="PSUM"))
        msp = ctx.enter_context(tc.tile_pool(name="msp", bufs=4, space="PSUM"))
        cps = ctx.enter_context(tc.tile_pool(name="cps", bufs=1, space="PSUM"))

        # outproj(tci): emitted interleaved into head 3's loop
        ops_state = {}

        def outproj_mm(tci, no):
            tsl = ts(tci, 128)
            op_t = msp.tile([128, 512], F32, tag="ms", name="op")
            for cc in range(2):
                nc.tensor.matmul(op_t, ctxT_t[cc][:, tsl], woT_t[cc][:, ts(no, 512)],
                                 start=(cc == 0), stop=(cc == 1))
            if no == 0:
                ops_state[tci] = ost_p.tile([128, E], BF16, tag="ost", name="ost")
            ost = ops_state[tci]
            nc.vector.tensor_copy(ost[:, ts(no, 512)], op_t)
            if no == 1:
                nc.sync.dma_start(out=outp_d[tsl, :], in_=ost)

        def outproj_steps(qb):
            # 8 paired-MM slots per score loop: 4 tci x 2 no
            return [(tci, no) for tci in range(qb * 4, qb * 4 + 4)
                    for no in range(2)]

        for h in range(NH):
            ch = h // 2
            pr = (h % 2) * 64
            mh_count = [0]
            for qb in range(NS):
                qsl = ts(qb, 512)
                # --- sT pass: K-stacked scores with fused -m-hat -> exp ---
                # one mh(h+1) step (or outproj MM for h==3) per kt so the PE
                # queue never stalls behind the DVE reduce chain
                steps = outproj_steps(qb - 1) if (h == 3 and qb > 0) else None
                pT = [ptp.tile([128, 512], BF16, tag=f"pt{i}", name=f"pt{i}")
                      for i in range(TC)]
                for kt in range(TC):
                    st = sps.tile([128, 512], F32, tag="st", name="st")
                    # kh·qh + kl·qh in one K=128 matmul (qh duplicated)
                    nc.tensor.matmul(st, khl[h][:, ts(kt, 128)], qhh[h][:, qsl],
                                     start=True, stop=False)
                    # kh·ql + ones·(-m-hat), K=65
                    nc.tensor.matmul(st, kha[h][:, ts(kt, 128)],
                                     qla[h][:, qsl], start=False, stop=True)
                    nc.scalar.activation(out=pT[kt], in_=st, func=Exp, scale=0.125)
                    if h < NH - 1 and qb < 3:
                        # 64 mh(h+1) steps over qb0..2 so the bounce lands
                        # during qb3, before head h+1 needs it
                        target = ((qb * TC + kt + 1) * 64) // (3 * TC)
                        while mh_count[0] < target:
                            mh_step(h + 1, mh_count[0], msp)
                            mh_count[0] += 1
                    elif steps is not None and kt % 2 == 0:
                        outproj_mm(*steps[kt // 2])
                if h < NH - 1 and qb == 2:
                    mh_finish(h + 1, msp)
                # --- PV with ones column ---
                cxa = cps.tile([VW, 512], F32, tag="cxa", name="cxa")
                for kt in range(TC):
                    nc.tensor.matmul(cxa, v16[kt][:, h * VW:(h + 1) * VW], pT[kt],
                                     start=(kt == 0), stop=(kt == TC - 1))
                # evacuate PSUM immediately so the next PV group never waits
                cxs = att.tile([VW, 512], F32, tag="cxs", name="cxs")
                nc.vector.tensor_copy(cxs, cxa)
                # --- normalize by Z (row 64) off the critical path ---
                zrow = att.tile([1, 512], F32, name="zrow")
                nc.vector.tensor_copy(zrow, cxs[64:65, :])
                z_bc = att.tile([64, 512], F32, name="z_bc")
                nc.gpsimd.partition_broadcast(z_bc, zrow, channels=64)
                rcp_bc = att.tile([64, 512], F32, name="rcp_bc")
                nc.vector.reciprocal_approx_fast(out=rcp_bc, in_=z_bc)
                nc.vector.tensor_mul(ctxT_t[ch][pr:pr + 64, qsl], cxs[0:64, :],
                                     rcp_bc)

        # ---------------- tail: last output-projection block ----------------
        for tci, no in outproj_steps(NS - 1):
            outproj_mm(tci, no)

    nc.compile()
    _CACHE[key] = nc
    return nc


def _split(a):
    h = a.astype(bf16)
    l = (a - h.astype(np.float32)).astype(bf16)
    return h, l


def _shard(inputs, lora):
    x = np.asarray(inputs["x"], np.float32)
    Wo = np.asarray(inputs["Wo"], np.float32)
    ident = np.eye(128, dtype=np.float32).astype(bf16)
    if lora:
        A_all = np.concatenate([np.asarray(inputs["Aq"], np.float32),
                                np.asarray(inputs["Ak"], np.float32),
                                np.asarray(inputs["Av"], np.float32)], axis=1)
        ah, al = _split(A_all)
    in_maps = []
    for core in range(8):
        b, hp = core // 4, core % 4
        o0 = hp * OL
        xT = np.ascontiguousarray(x[b].T)
        xh, xl = _split(xT)
        m = {"xth": xh, "xtl": xl, "ident": ident}
        for p in "qkv":
            W = np.asarray(inputs["W" + p], np.float32)
            Ws = np.ascontiguousarray(W[o0:o0 + OL, :].T)
            wh, wl = _split(Ws)
            m["w%sh" % p] = wh
            if p != "v":
                m["w%sl" % p] = wl
            if lora:
                B = np.asarray(inputs["B" + p], np.float32)[:, o0:o0 + OL] * 2.0
                m["b%sh" % p], m["b%sl" % p] = _split(B)
        m["woT"] = np.ascontiguousarray(Wo[:, o0:o0 + OL].T).astype(bf16)
        if lora:
            m["ah"], m["al"] = ah, al
        in_maps.append(m)
    return in_maps


def _run(inputs, trace=False, **kw):
    lora = not all(
        np.count_nonzero(np.asarray(inputs["B" + p])) == 0 for p in "qkv")
    nc = _build(lora)
    in_maps = _shard(inputs, lora)
    res = run_bass_kernel_spmd(nc, in_maps, core_ids=list(range(8)), trace=trace, **kw)
    bo = np.asarray(inputs["bo"], np.float32)
    parts = [res.results[c]["outp"].astype(np.float64) for c in range(8)]
    out = np.stack([sum(parts[0:4]), sum(parts[4:8])]) + bo.astype(np.float64)
    return out.astype(np.float32), res


def kernel(**inputs):
    out, _ = _run(inputs)
    return out
